# revision 4
# baseline (speedup 1.0000x reference)
"""Distributed ARMAConv kernel for 8 TRN2 NeuronCores (Bass/Tile).

Reference computation (N=16384 nodes, F=64 in-feats, C=32 channels,
K=2 stacks, T=2 iterations):
    for each stack k:  xbar = x
        for i in 0..1: xbar = relu(fltr @ (xbar @ w1) + x @ w2 + b)
    out = mean over stacks                                  -> [N, 32]

Strategy:
  - Row-shard fltr across 8 cores; core m holds fltr[rows_m, :] stored
    TRANSPOSED ([N, 2048], contraction-major) so TensorE tiles load as
    contiguous DMA.
  - Fuse the two independent stacks: Y = [xbar_k0 @ w1_k0 | xbar_k1 @ w1_k1]
    is [N, 64], so fltr is streamed from HBM only twice (once per
    iteration) instead of four times.  This is the memory roofline.
  - Iteration 0 needs no communication (x is replicated).  Between the
    iterations, one all-gather of Y1 = xbar1 @ w1 ([N, 64]), split into
    8 row-chunks so pass 2's compute overlaps the collective.
  - All big matmuls run transposed (out^T = Y^T @ fltr_m^T) so the moving
    operand streams 512 rows/instr, enabling the fast float32r PE path.
  - relu positive homogeneity folds the final stack-mean 0.5 scale into
    the pass-2 activation.
"""

import numpy as np

import concourse.mybir as mybir
import concourse.tile as tile
from concourse import bacc
from concourse.bass_utils import run_bass_kernel_spmd

N = 16384            # nodes
F = 64               # input features
C = 32               # channels per stack
C2 = 2 * C           # fused channels (2 stacks)
NCORES = 8
R = N // NCORES      # fltr rows per core (2048)
P = 128              # partitions
NKT = N // P         # K tiles per full pass (128)
RC = 4               # output row chunks per core
RCW = R // RC        # 512
GC = 8               # gather chunks
GW = R // GC         # 256 local rows per gather chunk
XCHUNK = 1024        # xT DMA chunk width

F32 = mybir.dt.float32
F32R = mybir.dt.float32r

_CACHE = {}


def _build():
    nc = bacc.Bacc(
        trn_type="TRN2", target_bir_lowering=False, debug=False,
        num_devices=NCORES,
    )
    fltrT_e = nc.dram_tensor("fltrt", [N, R], F32, kind="ExternalInput")
    xT_e = nc.dram_tensor("xt", [F, N], F32, kind="ExternalInput")
    xtm_e = nc.dram_tensor("xtm", [F, R], F32, kind="ExternalInput")
    w1i0_e = nc.dram_tensor("w1i0", [F, C2], F32, kind="ExternalInput")
    w1i1_e = nc.dram_tensor("w1i1", [C2, C2], F32, kind="ExternalInput")
    w2i0_e = nc.dram_tensor("w2i0", [F, C2], F32, kind="ExternalInput")
    w2i1_e = nc.dram_tensor("w2i1", [F, C2], F32, kind="ExternalInput")
    bi0_e = nc.dram_tensor("bi0", [C2, 1], F32, kind="ExternalInput")
    bi1h_e = nc.dram_tensor("bi1h", [C2, 1], F32, kind="ExternalInput")
    out_e = nc.dram_tensor("out", [C, R], F32, kind="ExternalOutput")

    RG = [list(range(NCORES))]

    with tile.TileContext(nc) as tc:
        with (
            tc.tile_pool(name="wpool", bufs=1) as wpool,
            tc.tile_pool(name="xcpool", bufs=3) as xcpool,
            tc.tile_pool(name="y0pool", bufs=1) as y0pool,
            tc.tile_pool(name="fpool", bufs=6) as fpool,
            tc.tile_pool(name="xbpool", bufs=4) as xbpool,
            tc.tile_pool(name="ylpool", bufs=4) as ylpool,
            tc.tile_pool(name="ygpool", bufs=3) as ygpool,
            tc.tile_pool(name="opool", bufs=1) as opool,
            tc.tile_pool(name="pacc", bufs=4, space="PSUM") as pacc,
            tc.tile_pool(name="psmall", bufs=2, space="PSUM") as psmall,
            tc.tile_pool(name="dram", bufs=8, space="DRAM") as dram,
        ):
            # resident small tensors
            w1i0 = wpool.tile([F, C2], F32)
            nc.sync.dma_start(w1i0[:], w1i0_e[:])
            w1i1 = wpool.tile([C2, C2], F32)  # block-diag [w1_k0i1, w1_k1i1]
            nc.sync.dma_start(w1i1[:], w1i1_e[:])
            w2i0 = wpool.tile([F, C2], F32R)
            nc.sync.dma_start(w2i0[:], w2i0_e[:].bitcast(F32R))
            w2i1 = wpool.tile([F, C2], F32R)
            nc.sync.dma_start(w2i1[:], w2i1_e[:].bitcast(F32R))
            bi0 = wpool.tile([C2, 1], F32)
            nc.sync.dma_start(bi0[:], bi0_e[:])
            bi1h = wpool.tile([C2, 1], F32)
            nc.sync.dma_start(bi1h[:], bi1h_e[:])
            xm = wpool.tile([F, R], F32R)
            nc.sync.dma_start(xm[:], xtm_e[:].bitcast(F32R))

            y0 = y0pool.tile([P, NKT, C2], F32R)  # node-major Y0 (lhsT tiles)

            # ---- pass 1 accumulators; skip term starts each group ----
            p1 = []
            for rc in range(RC):
                acc = pacc.tile([C2, RCW], F32, name=f"p1_{rc}", tag="acc")
                nc.tensor.matmul(
                    acc[:],
                    w2i0[:],
                    xm[:, rc * RCW:(rc + 1) * RCW],
                    start=True, stop=False,
                )
                p1.append(acc)

            # ---- Y0 = x @ [w1_k0i0 | w1_k1i0], node-major ----
            for g in range(N // XCHUNK):  # 16 groups of 8 kt
                xc = xcpool.tile([F, XCHUNK], F32, name="xc")
                nc.sync.dma_start(xc[:], xT_e[:, g * XCHUNK:(g + 1) * XCHUNK])
                ps0 = psmall.tile([P, 8, C2], F32, name="ps0", tag="ps0")
                for i in range(8):
                    nc.tensor.matmul(
                        ps0[:, i, :],
                        xc[:, i * P:(i + 1) * P],
                        w1i0[:],
                        start=True, stop=True,
                    )
                nc.vector.tensor_copy(y0[:, g * 8:(g + 1) * 8, :], ps0[:])

            # ---- pass 1 main: accumulate (fltr @ Y0)^T over all K ----
            for kt in range(NKT):
                ft = fpool.tile([P, R], F32R, name="ft")
                nc.sync.dma_start(ft[:], fltrT_e[kt * P:(kt + 1) * P, :].bitcast(F32R))
                for rc in range(RC):
                    nc.tensor.matmul(
                        p1[rc][:],
                        y0[:, kt, :],
                        ft[:, rc * RCW:(rc + 1) * RCW],
                        start=False, stop=(kt == NKT - 1),
                    )

            # ---- pass 1 epilogue: relu -> Y1 local -> chunked all-gather ----
            gouts = []
            for rc in range(RC):
                xb1 = xbpool.tile([C2, RCW], F32, name="xb1")
                nc.scalar.activation(
                    xb1[:], p1[rc][:], mybir.ActivationFunctionType.Relu,
                    bias=bi0[:], scale=1.0,
                )
                y1l = ylpool.tile([P, RC, C2], F32, name="y1l")
                for t in range(RC):  # node-subtiles of 128 within the chunk
                    psy = psmall.tile([P, C2], F32, name="psy", tag="psy")
                    nc.tensor.matmul(
                        psy[:],
                        xb1[:, t * P:(t + 1) * P],
                        w1i1[:],
                        start=True, stop=True,
                    )
                    nc.vector.tensor_copy(y1l[:, t, :], psy[:])
                for h in range(2):
                    gin = dram.tile([GW, C2], F32, name="gin", tag="gin")
                    nc.sync.dma_start(
                        gin[:].rearrange("(t p) ch -> p t ch", p=P),
                        y1l[:, 2 * h:2 * h + 2, :],
                    )
                    gout = dram.tile(
                        [NCORES * GW, C2], F32, name="gout", tag="gout",
                        addr_space="Shared",
                    )
                    nc.gpsimd.collective_compute(
                        "AllGather", mybir.AluOpType.bypass,
                        replica_groups=RG,
                        ins=[gin[:].opt()], outs=[gout[:].opt()],
                    )
                    gouts.append(gout)

            # ---- pass 2 accumulators ----
            p2 = []
            for rc in range(RC):
                acc = pacc.tile([C2, RCW], F32, name=f"p2_{rc}", tag="acc")
                nc.tensor.matmul(
                    acc[:],
                    w2i1[:],
                    xm[:, rc * RCW:(rc + 1) * RCW],
                    start=True, stop=False,
                )
                p2.append(acc)

            # ---- pass 2 main: consume gather chunks in order ----
            NB = NCORES * (GW // P)  # 16 K-blocks of 128 per gather chunk
            for c in range(GC):
                yg = ygpool.tile([P, NB, C2], F32R, name="yg")
                nc.sync.dma_start(
                    yg[:],
                    gouts[c][:].rearrange("(b p) ch -> p b ch", p=P).bitcast(F32R),
                )
                for b in range(NB):
                    j, t = b // 2, b % 2
                    base = j * R + c * GW + t * P
                    ft = fpool.tile([P, R], F32R, name="ft")
                    nc.sync.dma_start(ft[:], fltrT_e[base:base + P, :].bitcast(F32R))
                    last = (c == GC - 1) and (b == NB - 1)
                    for rc in range(RC):
                        nc.tensor.matmul(
                            p2[rc][:],
                            yg[:, b, :],
                            ft[:, rc * RCW:(rc + 1) * RCW],
                            start=False, stop=last,
                        )

            # ---- pass 2 epilogue: relu(0.5 z + 0.5 b), stack mean ----
            outT = opool.tile([C, R], F32)
            for rc in range(RC):
                xb2 = xbpool.tile([C2, RCW], F32, name="xb2")
                nc.scalar.activation(
                    xb2[:], p2[rc][:], mybir.ActivationFunctionType.Relu,
                    bias=bi1h[:], scale=0.5,
                )
                # partition-shift stack-1 half to base 0 (DMA), then add
                xs = xbpool.tile([C, RCW], F32, name="xs")
                nc.sync.dma_start(xs[:], xb2[C:C2, :])
                nc.vector.tensor_add(
                    outT[:, rc * RCW:(rc + 1) * RCW],
                    xb2[0:C, :], xs[:],
                )
            nc.sync.dma_start(out_e[:], outT[:])

    nc.compile()
    return nc


def kernel(**inputs):
    x = np.ascontiguousarray(np.asarray(inputs["x"], dtype=np.float32))
    fltr = np.ascontiguousarray(np.asarray(inputs["fltr"], dtype=np.float32))

    def cat(a, b, axis=1):
        return np.ascontiguousarray(
            np.concatenate(
                [np.asarray(a, np.float32), np.asarray(b, np.float32)],
                axis=axis,
            )
        )

    w1i0 = cat(inputs["k0i0_w1"], inputs["k1i0_w1"])
    w1i1 = np.zeros((C2, C2), dtype=np.float32)
    w1i1[0:C, 0:C] = np.asarray(inputs["k0i1_w1"], np.float32)
    w1i1[C:C2, C:C2] = np.asarray(inputs["k1i1_w1"], np.float32)
    w2i0 = cat(inputs["k0i0_w2"], inputs["k1i0_w2"])
    w2i1 = cat(inputs["k0i1_w2"], inputs["k1i1_w2"])
    bi0 = cat(inputs["k0i0_b"], inputs["k1i0_b"], axis=0)[:, None]
    bi1h = 0.5 * cat(inputs["k0i1_b"], inputs["k1i1_b"], axis=0)[:, None]
    bi1h = np.ascontiguousarray(bi1h)
    xT = np.ascontiguousarray(x.T)

    if "nc" not in _CACHE:
        _CACHE["nc"] = _build()
    nc = _CACHE["nc"]

    in_maps = []
    for m in range(NCORES):
        rows = slice(m * R, (m + 1) * R)
        in_maps.append({
            "fltrt": np.ascontiguousarray(fltr[rows, :].T),
            "xt": xT,
            "xtm": np.ascontiguousarray(x[rows, :].T),
            "w1i0": w1i0, "w1i1": w1i1, "w2i0": w2i0, "w2i1": w2i1,
            "bi0": bi0, "bi1h": bi1h,
        })

    import os
    trace = os.environ.get("ARMA_TRACE") == "1"
    res = run_bass_kernel_spmd(
        nc, in_maps, core_ids=list(range(NCORES)), trace=trace,
    )
    _CACHE["last_results"] = res
    out = np.concatenate(
        [np.asarray(res.results[m]["out"]).T for m in range(NCORES)], axis=0
    )
    return out
